# revision 51
# baseline (speedup 1.0000x reference)
"""GPT2 self-attention on 8 trn2 NeuronCores (tensor-parallel).

Sharding: core c handles batch b = c//4 and head-group g = c%4
(4 of 16 heads = 256 of 1024 dims).

Per core, qc-major (512-token query chunks), head pairs sequential:
  1. QK^T projection per chunk: [512 qk-dims, 512 tokens] = wqk^T @ x
  2. V projection per chunk:    [512 tokens, 256 v-dims]  = x @ wv
  3. Attention per (pair, chunk), keys on PSUM partitions:
       S^T[k, q] for both heads of the pair in one 2-bank PSUM tile
       exp(S/8) merged over both heads on ACT -> probs bf16
       causal diag handled by multiplying probs with a keep-mask (DVE)
       flipped AV: O[q, 65] += probs[:, qt].T @ [V | 1]  (N=65 on PE)
       per-partition reciprocal + tensor_scalar normalize -> O_norm bf16
       PE transpose [128q, 128d] -> O^T chunk
  4. Three packed AllGathers (group of 4) over token ranges
     (0:1024, 1024:1536, 1536:2048), both pairs packed per gather.
  5. Output projection per gathered token tile: z[128t, 256] via 8
     k-tiles of O^T_full against a host-sliced w_out column shard.

Host only reorders/slices/casts inputs (x^T, weight slices, bf16) and
places the 8 per-core z column-chunks into [B, S, D]. b_qkv/b_out are
zeros by the problem spec and folded out. Matmuls run bf16 with fp32
PSUM accumulation.
"""

import numpy as np
import ml_dtypes
from contextlib import ExitStack

B, S, D, H = 2, 2048, 1024, 16
HD = 64            # head dim
NCORES = 8
HPC = 4            # heads per core
GD = HPC * HD      # 256 dims per core group
QW = 512           # query-chunk width
NQC = S // QW      # 4 query chunks
NKT = S // 128     # 16 key tiles

# per gather: (start_tile, end_tile, head-pairs) in 128-token tiles
GATHERS = [(0, 8, (0, 1)), (8, 12, (0, 1)), (12, 16, (0, 1))]

_CACHE = {}


def _build_program():
    import concourse.tile as tile
    from concourse import bacc, mybir

    bf16 = mybir.dt.bfloat16
    f32 = mybir.dt.float32

    nc = bacc.Bacc("TRN2", target_bir_lowering=False, debug=False,
                   num_devices=NCORES)

    xt = nc.dram_tensor("xt", [D, S], bf16, kind="ExternalInput").ap()
    wqk = nc.dram_tensor("wqk", [D, 2 * GD], bf16, kind="ExternalInput").ap()
    wv = nc.dram_tensor("wv", [D, GD], bf16, kind="ExternalInput").ap()
    wout = nc.dram_tensor("wout", [D, GD], bf16, kind="ExternalInput").ap()
    keep2 = nc.dram_tensor("keep2", [128, 256], bf16, kind="ExternalInput").ap()
    ident = nc.dram_tensor("ident", [128, 128], bf16, kind="ExternalInput").ap()
    z_out = nc.dram_tensor("z", [S, GD], f32, kind="ExternalOutput").ap()

    KD = D // 128           # 8 contraction tiles over d_model

    with tile.TileContext(nc) as tc, ExitStack() as ctx:
        persist = ctx.enter_context(tc.tile_pool(name="persist", bufs=1))
        # PSUM budget: sc(2x2) + ot(2x1) + proj(2x1) = 8 banks
        sc_ps = ctx.enter_context(tc.tile_pool(name="sc_ps", bufs=2, space="PSUM"))
        ot_ps = ctx.enter_context(tc.tile_pool(name="ot_ps", bufs=2, space="PSUM"))
        pj_ps = ctx.enter_context(tc.tile_pool(name="pj_ps", bufs=2, space="PSUM"))
        on_pool = ctx.enter_context(tc.tile_pool(name="on_pool", bufs=6))
        rec_pool = ctx.enter_context(tc.tile_pool(name="rec_pool", bufs=4))
        zs_pool = ctx.enter_context(tc.tile_pool(name="zs_pool", bufs=3))
        dram_pool = ctx.enter_context(tc.tile_pool(name="dram_pool", bufs=1, space="DRAM"))

        xt_sb = [persist.tile([128, S], bf16, tag=f"xt{k}", name=f"xt{k}") for k in range(KD)]
        zev_sb = [persist.tile([128, GD], f32, tag=f"zev{i}", name=f"zev{i}")
                  for i in range(4)]
        wqk_sb = [persist.tile([128, 2 * GD], bf16, tag=f"wqk{k}", name=f"wqk{k}") for k in range(KD)]
        wv_sb = [persist.tile([128, GD], bf16, tag=f"wv{k}", name=f"wv{k}") for k in range(KD)]
        wout_sb = [persist.tile([128, GD], bf16, tag=f"wout{k}", name=f"wout{k}") for k in range(KD)]
        keep_sb = persist.tile([128, 2, 128], bf16, tag="keep", name="keep_sb")
        ident_sb = persist.tile([128, 128], bf16, tag="ident", name="ident_sb")
        qkt_sb = [persist.tile([128, S], bf16, tag=f"qkt{m}", name=f"qkt{m}") for m in range(4)]
        v_sb = [persist.tile([128, HPC, HD + 1], bf16, tag=f"v{t}", name=f"v{t}") for t in range(NKT)]
        # probs for one whole chunk (all key tiles), double-buffered by pair
        pr_sb = [[persist.tile([128, 2, QW], bf16, tag=f"pr{pp}_{kt}",
                               name=f"pr{pp}_{kt}") for kt in range(NKT)]
                 for pp in range(2)]
        otT_sb = [persist.tile([128, S], bf16, tag=f"otT{p}", name=f"otT{p}") for p in range(2)]
        otf_sb = [persist.tile([128, S], bf16, tag=f"otf{k}", name=f"otf{k}") for k in range(KD)]

        # ---- initial loads, spread across DMA queues; first-needed cols
        # (chunk 0) land first so the QK^T projection starts early ----
        for k in range(KD):
            eng = (nc.sync, nc.scalar)[k % 2]
            eng.dma_start(out=xt_sb[k][:, 0:QW], in_=xt[k * 128:(k + 1) * 128, 0:QW])
        for k in range(KD):
            nc.gpsimd.dma_start(out=wqk_sb[k][:], in_=wqk[k * 128:(k + 1) * 128, :])
        for k in range(KD):
            # keep the scalar queue free for exp from here on
            nc.sync.dma_start(out=xt_sb[k][:, QW:S], in_=xt[k * 128:(k + 1) * 128, QW:S])
        for k in range(KD):
            nc.gpsimd.dma_start(out=wv_sb[k][:], in_=wv[k * 128:(k + 1) * 128, :])
        nc.gpsimd.dma_start(
            out=keep_sb[:], in_=keep2[:].rearrange("p (h q) -> p h q", h=2))
        nc.gpsimd.dma_start(out=ident_sb[:], in_=ident[:])
        for k in range(KD):
            nc.gpsimd.dma_start(out=wout_sb[k][:], in_=wout[k * 128:(k + 1) * 128, :])

        # ---- filler machinery: projection matmuls injected into the PE
        # stream between attention ops to hide exp latency ----
        filler_q = []  # list of thunks, each issuing ONE PE matmul (+ tail)

        def add_qkt_chunk(m, qc):
            """QK^T projection: out [128 qk-dims, 512 tokens] for chunk qc."""
            st = {}

            def step(k, st=st, m=m, qc=qc):
                if k == 0:
                    st["ps"] = pj_ps.tile([128, QW], f32, tag="pj", name="qkt_ps")
                nc.tensor.matmul(
                    st["ps"][:],
                    wqk_sb[k][:, m * 128:(m + 1) * 128],
                    xt_sb[k][:, qc * QW:(qc + 1) * QW],
                    start=(k == 0), stop=(k == KD - 1),
                )
                if k == KD - 1:
                    nc.vector.tensor_copy(
                        qkt_sb[m][:, qc * QW:(qc + 1) * QW], st["ps"][:])

            for k in range(KD):
                filler_q.append(lambda k=k: step(k))

        def add_v_tile(t):
            """V projection: out [128 tokens, 256 v-dims] for token tile t."""
            st = {}

            def step(k, st=st, t=t):
                if k == 0:
                    st["ps"] = pj_ps.tile([128, GD], f32, tag="pj", name="v_ps")
                nc.tensor.matmul(
                    st["ps"][:, 0:GD],
                    xt_sb[k][:, t * 128:(t + 1) * 128],
                    wv_sb[k][:],
                    start=(k == 0), stop=(k == KD - 1),
                )
                if k == KD - 1:
                    nc.vector.tensor_copy(
                        v_sb[t][:, :, 0:HD],
                        st["ps"][:, 0:GD].rearrange("p (h d) -> p h d", h=HPC),
                    )
                    nc.vector.memset(v_sb[t][:, :, HD:HD + 1], 1.0)

            for k in range(KD):
                filler_q.append(lambda k=k: step(k))

        def add_zproj(mt):
            """Out-proj for token tile mt: z[128t, 256] over 8 k-tiles."""
            st = {}

            def step(k, st=st, mt=mt):
                if k == 0:
                    st["ps"] = pj_ps.tile([128, GD], f32, tag="pj", name="z_ps")
                nc.tensor.matmul(
                    st["ps"][:, 0:GD],
                    otf_sb[k][:, mt * 128:(mt + 1) * 128],
                    wout_sb[k][:],
                    start=(k == 0), stop=(k == KD - 1),
                )
                if k == KD - 1:
                    zrow = zs_pool.tile([128, GD], f32, tag="zrow", name="zrow_t")
                    nc.vector.tensor_copy(zrow[:], st["ps"][:, 0:GD])
                    # scalar queue is exp-free by the time the last tiles
                    # store; gpsimd still carries the gather-2 fanout
                    eng = nc.scalar if mt >= 12 else nc.gpsimd
                    eng.dma_start(
                        out=z_out[mt * 128:(mt + 1) * 128, :], in_=zrow[:])

            for k in range(KD):
                filler_q.append(lambda k=k: step(k))

        def fill(n):
            for _ in range(n):
                if filler_q:
                    filler_q.pop(0)()

        def drain_fillers():
            while filler_q:
                filler_q.pop(0)()

        # ---- gathers ----
        ag_in = [dram_pool.tile([128, len(ps_), 128 * (t1 - t0)], bf16,
                                tag=f"agin{gi}", name=f"agin{gi}")
                 for gi, (t0, t1, ps_) in enumerate(GATHERS)]
        ag_out = [dram_pool.tile([512, len(ps_), 128 * (t1 - t0)], bf16,
                                 tag=f"agout{gi}", name=f"agout{gi}")
                  for gi, (t0, t1, ps_) in enumerate(GATHERS)]
        # (qc, pair) -> (gather idx, token offset in range, pair slot)
        AG_SLOT = {(0, 0): (0, 0, 0), (0, 1): (0, 0, 1),
                   (1, 0): (0, QW, 0), (1, 1): (0, QW, 1),
                   (2, 0): (1, 0, 0), (2, 1): (1, 0, 1),
                   (3, 0): (2, 0, 0), (3, 1): (2, 0, 1)}

        # ---- attention split into scores and AV phases; pair p's AV
        # drains during pair p+1's scores so its exps are already done
        # (PE wait-queue is 4 deep: a blocked AV group jams the stream) ----
        def scores_phase(pair, qc):
            qstart = qc * QW
            nkt = 4 * (qc + 1)
            prs = pr_sb[pair]
            for kt in range(nkt):
                j = kt - 4 * qc           # diag sub-tile index if >= 0
                qoff = max(0, 128 * j)
                sp = sc_ps.tile([128, 2, QW], f32, tag="sc", name="sc_t")
                for hh in range(2):
                    base = 64 * hh
                    nc.tensor.matmul(
                        sp[:, hh, qoff:QW],
                        qkt_sb[2 + pair][base:base + 64, kt * 128:(kt + 1) * 128],
                        qkt_sb[pair][base:base + 64, qstart + qoff:qstart + QW],
                        start=True, stop=True,
                    )
                nc.scalar.activation(
                    prs[kt][:, :, qoff:QW], sp[:, :, qoff:QW],
                    mybir.ActivationFunctionType.Exp,
                    scale=0.125,
                )
                if j >= 0:
                    nc.vector.tensor_mul(
                        prs[kt][:, :, qoff:qoff + 128],
                        prs[kt][:, :, qoff:qoff + 128],
                        keep_sb[:],
                    )
                yield

        def av_phase(pair, qc):
            qstart = qc * QW
            prs = pr_sb[pair]
            otp = [ot_ps.tile([128, 4, HD + 1], f32, tag="ot", name="otp_t")
                   for _ in range(2)]
            rec = rec_pool.tile([128, 2, 4], f32, tag="rec", name="rec_t")
            gi, loc, sl = AG_SLOT[(qc, pair)]
            for l in range(4):
                qt = 4 * qc + l
                for hh in range(2):
                    h = 2 * pair + hh
                    for kt in range(qt + 1):
                        nc.tensor.matmul(
                            otp[hh][:, l, :],
                            prs[kt][:, hh, l * 128:(l + 1) * 128],
                            v_sb[kt][:, h, :],
                            start=(kt == 0), stop=(kt == qt),
                        )
                    nc.vector.reciprocal(rec[:, hh, l:l + 1],
                                         otp[hh][:, l, HD:HD + 1])
                onorm = on_pool.tile([128, 128], bf16, tag="on", name="on_t")
                for hh in range(2):
                    nc.vector.tensor_scalar_mul(
                        onorm[:, 64 * hh:64 * hh + 64],
                        otp[hh][:, l, 0:HD],
                        rec[:, hh, l:l + 1],
                    )
                tp = pj_ps.tile([128, 128], bf16, tag="pj", name="tp_t")
                nc.tensor.transpose(tp[:], onorm[:], ident_sb[:])
                nc.vector.tensor_copy(
                    otT_sb[pair][:, qstart + 128 * l:qstart + 128 * (l + 1)],
                    tp[:],
                )
                nc.sync.dma_start(
                    out=ag_in[gi][:, sl, loc + 128 * l:loc + 128 * (l + 1)],
                    in_=otT_sb[pair][:, qstart + 128 * l:qstart + 128 * (l + 1)],
                )
                yield

        def collective(gi, eng):
            # issued on a queue whose head isn't blocked by pending waits
            eng.collective_compute(
                "AllGather",
                mybir.AluOpType.bypass,
                replica_groups=[[0, 1, 2, 3], [4, 5, 6, 7]],
                ins=[ag_in[gi][:].opt()],
                outs=[ag_out[gi][:].opt()],
            )

        def fanout(gi, engs):
            t0, t1, ps_ = GATHERS[gi]
            i = 0
            for r in range(4):
                for sl, pair in enumerate(ps_):
                    engs[i % len(engs)].dma_start(
                        out=otf_sb[2 * r + pair][:, 128 * t0:128 * t1],
                        in_=ag_out[gi][128 * r:128 * (r + 1), sl, :],
                    )
                    i += 1

        def gather(gi):
            collective(gi, nc.gpsimd)
            fanout(gi, [nc.gpsimd])

        # ---- main schedule ----
        add_qkt_chunk(0, 0); add_qkt_chunk(1, 0)
        add_qkt_chunk(2, 0); add_qkt_chunk(3, 0)
        for t in range(4):
            add_v_tile(t)
        drain_fillers()

        post_av_hook = {(1, 1): lambda: gather(0), (1, 2): lambda: gather(1)}
        prev_av = None
        prev_key = None
        for qc in range(NQC):
            for pair in range(2):
                if pair == 0 and qc + 1 < NQC:
                    for m in range(4):
                        add_qkt_chunk(m, qc + 1)
                    for t in range(4 * qc + 4, 4 * qc + 8):
                        add_v_tile(t)
                if (pair, qc) == (1, 2):
                    for mt in range(8):
                        add_zproj(mt)
                if (pair, qc) == (1, 3):
                    for mt in range(8, 12):
                        add_zproj(mt)
                nkt = 4 * (qc + 1)
                av_every = max(1, nkt // 5)
                i = 0
                for _ in scores_phase(pair, qc):
                    fill(2)
                    i += 1
                    if prev_av is not None and i % av_every == 0:
                        fill(1)
                        if next(prev_av, "end") == "end":
                            prev_av = None
                            if prev_key in post_av_hook:
                                post_av_hook[prev_key]()
                if prev_av is not None:
                    for _ in prev_av:
                        fill(3)
                    if prev_key in post_av_hook:
                        post_av_hook[prev_key]()
                prev_av = av_phase(pair, qc)
                prev_key = (pair, qc)
        for _ in prev_av:
            fill(8)
        drain_fillers()
        collective(2, nc.gpsimd)
        fanout(2, [nc.gpsimd, nc.scalar, nc.sync])
        # keep the PE p-state warm through the collective+fanout window so
        # the final out-proj runs at full clock (idle resets the ramp)
        for d in range(72):
            dm = sc_ps.tile([128, 2, QW], f32, tag="sc", name="warm_t")
            nc.tensor.matmul(dm[:, 0, :], ident_sb[:], qkt_sb[0][:, 0:QW],
                             start=True, stop=True)
        for mt in range(12, 16):
            ps = pj_ps.tile([128, GD], f32, tag="pj", name="zl_ps")
            for k in range(KD):
                nc.tensor.matmul(
                    ps[:, 0:GD],
                    otf_sb[k][:, mt * 128:(mt + 1) * 128],
                    wout_sb[k][:],
                    start=(k == 0), stop=(k == KD - 1),
                )
            zrow = zs_pool.tile([128, GD], f32, tag="zrow", name="zrow_t")
            nc.vector.tensor_copy(zrow[:], ps[:, 0:GD])
            eng = (nc.scalar, nc.sync)[mt % 2]
            eng.dma_start(
                out=z_out[mt * 128:(mt + 1) * 128, :], in_=zrow[:])

    nc.compile()
    return nc


def _get_program():
    if "nc" not in _CACHE:
        _CACHE["nc"] = _build_program()
    return _CACHE["nc"]


def _make_in_maps(x, w_qkv, w_out):
    bf = ml_dtypes.bfloat16
    keep = np.triu(np.ones((128, 128), dtype=np.float32))
    keep2 = np.concatenate([keep, keep], axis=1).astype(bf)
    ident = np.eye(128, dtype=np.float32).astype(bf)
    in_maps = []
    for c in range(NCORES):
        b, g = c // 4, c % 4
        cs = slice(GD * g, GD * (g + 1))
        xt = np.ascontiguousarray(x[b].T).astype(bf)
        wqk = np.concatenate(
            [w_qkv[:, cs], w_qkv[:, D + GD * g:D + GD * (g + 1)]], axis=1
        ).astype(bf)
        wv = np.ascontiguousarray(w_qkv[:, 2 * D + GD * g:2 * D + GD * (g + 1)]).astype(bf)
        wo = np.ascontiguousarray(w_out[:, cs]).astype(bf)
        in_maps.append(
            {"xt": xt, "wqk": wqk, "wv": wv, "wout": wo,
             "keep2": keep2, "ident": ident})
    return in_maps


def kernel(x, w_qkv, b_qkv, w_out, b_out):
    from concourse.bass_utils import run_bass_kernel_spmd

    x = np.asarray(x, dtype=np.float32)
    w_qkv = np.asarray(w_qkv, dtype=np.float32)
    w_out = np.asarray(w_out, dtype=np.float32)

    nc = _get_program()
    in_maps = _make_in_maps(x, w_qkv, w_out)
    res = run_bass_kernel_spmd(nc, in_maps, list(range(NCORES))).results

    out = np.empty((B, S, D), dtype=np.float32)
    for c in range(NCORES):
        b, g = c // 4, c % 4
        out[b, :, GD * g:GD * (g + 1)] = res[c]["z"]
    return out
